# revision 3
# baseline (speedup 1.0000x reference)
"""Trainium2 Bass kernel for nn_CrossMarketCompoundEmbedding.

Output[i] = concat(price_w[0], size_w[0], exchange_w[i%3], pair_w[i%4])
for i in [0, 65536) -> [65536, 512] f32.

The output is periodic with period lcm(3,4)=12 rows (one "super-row" of
12*512 f32 = 24 KiB). Per core (8 cores, 8192 rows each = 16 MiB) the
kernel is pure HBM-write bandwidth: stage one super-row per SBUF
partition (all 128 partitions identical content, phase-shifted per core
on the host), then blast it to the output DRAM with a few large DMAs.

Layout: SBUF tile [128, 6144] f32, partition p supplies output rows
[c*1536 + p*12, c*1536 + p*12 + 12) of each 1536-row chunk. Since
1536 % 12 == 0 the same tile serves every chunk.
"""

import numpy as np

EMBED_DIM = 512
D4 = EMBED_DIM // 4
NUM_FEATURES = 65536
N_CORES = 8
ROWS_PER_CORE = NUM_FEATURES // N_CORES  # 8192
PERIOD = 12                              # lcm(3, 4)
SUPER = PERIOD * EMBED_DIM               # 6144 f32 per partition
CHUNK_ROWS = 128 * PERIOD                # 1536 rows per full-tile store
N_FULL = ROWS_PER_CORE // CHUNK_ROWS     # 5 full chunks -> 7680 rows
REM_ROWS = ROWS_PER_CORE - N_FULL * CHUNK_ROWS   # 512
REM_PARTS = REM_ROWS // PERIOD           # 42 partitions -> 504 rows
TAIL_ROWS = REM_ROWS - REM_PARTS * PERIOD  # 8 rows

_CACHE = {}

# test.py hooks (harness ignores these)
TRACE = False
LAST_EXEC_NS = None
LAST_RESULTS = None


def _build_program():
    import concourse.bass as bass
    import concourse.bacc as bacc
    import concourse.mybir as mybir

    nc = bacc.Bacc(
        "TRN2",
        target_bir_lowering=False,
        debug=False,
        enable_asserts=False,
        num_devices=N_CORES,
    )
    f32 = mybir.dt.float32
    block = nc.dram_tensor("block", [128, SUPER], f32, kind="ExternalInput").ap()
    out = nc.dram_tensor("out", [ROWS_PER_CORE, EMBED_DIM], f32, kind="ExternalOutput").ap()

    with (
        nc.sbuf_tensor("pat", [128, SUPER], f32) as t,
        nc.semaphore("dma_sem") as dma_sem,
        nc.Block() as blk,
    ):
        @blk.sync
        def _(sync):
            sync.dma_start(t[:, :], block[:, :]).then_inc(dma_sem, 16)
            sync.wait_ge(dma_sem, 16)
            # Remainder first: its port-imbalanced descriptors hide behind
            # the balanced chunk stores that follow in the same ring.
            rem = out[N_FULL * CHUNK_ROWS : N_FULL * CHUNK_ROWS + REM_PARTS * PERIOD]
            rem = rem.rearrange("(p r) d -> p (r d)", r=PERIOD)  # [42, SUPER]
            sync.dma_start(rem, t[0 : 3 * REM_PARTS : 3, :]).then_inc(dma_sem, 16)
            tail = out[ROWS_PER_CORE - TAIL_ROWS :].rearrange(
                "(p r) d -> p (r d)", p=1
            )  # [1, TAIL_ROWS*512]
            sync.dma_start(tail, t[:1, : TAIL_ROWS * EMBED_DIM]).then_inc(dma_sem, 16)
            full = out[: N_FULL * CHUNK_ROWS].rearrange(
                "(k p r) d -> k p (r d)", p=128, r=PERIOD
            )  # [N_FULL, 128, SUPER]
            for k in range(N_FULL):
                sync.dma_start(full[k], t[:, :]).then_inc(dma_sem, 16)
            sync.wait_ge(dma_sem, 16 * (3 + N_FULL))
    nc.compile()
    return nc


def _get_program():
    if "nc" not in _CACHE:
        _CACHE["nc"] = _build_program()
    return _CACHE["nc"]


def _host_blocks(price_w, size_w, exchange_w, pair_w):
    """Per-core [128, SUPER] f32 pattern blocks (all partitions identical)."""
    idx = np.arange(PERIOD)
    row12 = np.concatenate(
        [
            np.broadcast_to(price_w[0], (PERIOD, D4)),
            np.broadcast_to(size_w[0], (PERIOD, D4)),
            exchange_w[idx % 3],
            pair_w[idx % 4],
        ],
        axis=-1,
    ).astype(np.float32)  # [12, 512]
    blocks = []
    for c in range(N_CORES):
        base = c * ROWS_PER_CORE
        s = row12[(base + idx) % PERIOD].reshape(-1)  # [SUPER]
        blocks.append(np.ascontiguousarray(np.broadcast_to(s, (128, SUPER))))
    return blocks


def kernel(num_features, price_w, size_w, exchange_w, pair_w):
    global LAST_EXEC_NS, LAST_RESULTS
    from concourse.bass_utils import run_bass_kernel_spmd

    assert int(num_features) == NUM_FEATURES
    price_w = np.asarray(price_w, dtype=np.float32)
    size_w = np.asarray(size_w, dtype=np.float32)
    exchange_w = np.asarray(exchange_w, dtype=np.float32)
    pair_w = np.asarray(pair_w, dtype=np.float32)

    nc = _get_program()
    in_maps = [{"block": b} for b in _host_blocks(price_w, size_w, exchange_w, pair_w)]
    res = run_bass_kernel_spmd(nc, in_maps, list(range(N_CORES)), trace=TRACE)
    LAST_EXEC_NS = res.exec_time_ns
    LAST_RESULTS = res
    return np.concatenate([res.results[c]["out"] for c in range(N_CORES)], axis=0)


# revision 4
# speedup vs baseline: 1.0656x; 1.0656x over previous
"""Trainium2 Bass kernel for nn_CrossMarketCompoundEmbedding.

Output[i] = concat(price_w[0], size_w[0], exchange_w[i%3], pair_w[i%4])
for i in [0, 65536) -> [65536, 512] f32.

The output is periodic with period lcm(3,4)=12 rows (one "super-row" of
12*512 f32 = 24 KiB). Per core (8 cores, 8192 rows each = 16 MiB) the
kernel is pure HBM-write bandwidth: stage one super-row per SBUF
partition (all 128 partitions identical content, phase-shifted per core
on the host), then blast it to the output DRAM with a few large DMAs.

Layout: SBUF tile [128, 6144] f32, partition p supplies output rows
[c*1536 + p*12, c*1536 + p*12 + 12) of each 1536-row chunk. Since
1536 % 12 == 0 the same tile serves every chunk.
"""

import numpy as np

EMBED_DIM = 512
D4 = EMBED_DIM // 4
NUM_FEATURES = 65536
N_CORES = 8
ROWS_PER_CORE = NUM_FEATURES // N_CORES  # 8192
PERIOD = 12                              # lcm(3, 4)
SUPER = PERIOD * EMBED_DIM               # 6144 f32 per partition
CHUNK_ROWS = 128 * PERIOD                # 1536 rows per full-tile store
N_FULL = ROWS_PER_CORE // CHUNK_ROWS     # 5 full chunks -> 7680 rows
REM_ROWS = ROWS_PER_CORE - N_FULL * CHUNK_ROWS   # 512
REM_PARTS = REM_ROWS // PERIOD           # 42 partitions -> 504 rows
TAIL_ROWS = REM_ROWS - REM_PARTS * PERIOD  # 8 rows

_CACHE = {}

# test.py hooks (harness ignores these)
TRACE = False
LAST_EXEC_NS = None
LAST_RESULTS = None


def _build_program():
    import concourse.bass as bass
    import concourse.bacc as bacc
    import concourse.mybir as mybir

    nc = bacc.Bacc(
        "TRN2",
        target_bir_lowering=False,
        debug=False,
        enable_asserts=False,
        num_devices=N_CORES,
    )
    f32 = mybir.dt.float32
    block = nc.dram_tensor("block", [128, SUPER], f32, kind="ExternalInput").ap()
    out = nc.dram_tensor("out", [ROWS_PER_CORE, EMBED_DIM], f32, kind="ExternalOutput").ap()

    with (
        nc.sbuf_tensor("pat", [128, SUPER], f32) as t,
        nc.semaphore("dma_sem") as dma_sem,
        nc.semaphore("st_sem") as st_sem,
        nc.Block() as blk,
    ):
        rem = out[N_FULL * CHUNK_ROWS : N_FULL * CHUNK_ROWS + REM_PARTS * PERIOD]
        rem = rem.rearrange("(p r) d -> p (r d)", r=PERIOD)  # [42, SUPER]
        tail = out[ROWS_PER_CORE - TAIL_ROWS :].rearrange(
            "(p r) d -> p (r d)", p=1
        )  # [1, TAIL_ROWS*512]
        full = out[: N_FULL * CHUNK_ROWS].rearrange(
            "(k p r) d -> k p (r d)", p=128, r=PERIOD
        )  # [N_FULL, 128, SUPER]

        # Two HWDGE rings (SP + ACT) halve descriptor-feed latency and stop
        # engine 15 from starving on a single ring's round-robin.
        @blk.sync
        def _(sync):
            sync.dma_start(t[:, :], block[:, :]).then_inc(dma_sem, 16)
            sync.wait_ge(dma_sem, 16)
            for k in (0, 2, 4):
                sync.dma_start(full[k], t[:, :]).then_inc(st_sem, 16)
            sync.dma_start(tail, t[:1, : TAIL_ROWS * EMBED_DIM]).then_inc(st_sem, 16)
            sync.wait_ge(st_sem, 16 * 7)

        @blk.scalar
        def _(scalar):
            scalar.wait_ge(dma_sem, 16)
            for k in (1, 3):
                scalar.dma_start(full[k], t[:, :]).then_inc(st_sem, 16)
            scalar.dma_start(rem, t[0 : 3 * REM_PARTS : 3, :]).then_inc(st_sem, 16)
    nc.compile()
    return nc


def _get_program():
    if "nc" not in _CACHE:
        _CACHE["nc"] = _build_program()
    return _CACHE["nc"]


def _host_blocks(price_w, size_w, exchange_w, pair_w):
    """Per-core [128, SUPER] f32 pattern blocks (all partitions identical)."""
    idx = np.arange(PERIOD)
    row12 = np.concatenate(
        [
            np.broadcast_to(price_w[0], (PERIOD, D4)),
            np.broadcast_to(size_w[0], (PERIOD, D4)),
            exchange_w[idx % 3],
            pair_w[idx % 4],
        ],
        axis=-1,
    ).astype(np.float32)  # [12, 512]
    blocks = []
    for c in range(N_CORES):
        base = c * ROWS_PER_CORE
        s = row12[(base + idx) % PERIOD].reshape(-1)  # [SUPER]
        blocks.append(np.ascontiguousarray(np.broadcast_to(s, (128, SUPER))))
    return blocks


def kernel(num_features, price_w, size_w, exchange_w, pair_w):
    global LAST_EXEC_NS, LAST_RESULTS
    from concourse.bass_utils import run_bass_kernel_spmd

    assert int(num_features) == NUM_FEATURES
    price_w = np.asarray(price_w, dtype=np.float32)
    size_w = np.asarray(size_w, dtype=np.float32)
    exchange_w = np.asarray(exchange_w, dtype=np.float32)
    pair_w = np.asarray(pair_w, dtype=np.float32)

    nc = _get_program()
    in_maps = [{"block": b} for b in _host_blocks(price_w, size_w, exchange_w, pair_w)]
    res = run_bass_kernel_spmd(nc, in_maps, list(range(N_CORES)), trace=TRACE)
    LAST_EXEC_NS = res.exec_time_ns
    LAST_RESULTS = res
    return np.concatenate([res.results[c]["out"] for c in range(N_CORES)], axis=0)


# revision 5
# speedup vs baseline: 1.2171x; 1.1422x over previous
"""Trainium2 Bass kernel for nn_CrossMarketCompoundEmbedding.

Output[i] = concat(price_w[0], size_w[0], exchange_w[i%3], pair_w[i%4])
for i in [0, 65536) -> [65536, 512] f32.

The output is periodic with period lcm(3,4)=12 rows (one "super-row" of
12*512 f32 = 24 KiB). Per core (8 cores, 8192 rows each = 16 MiB) the
kernel is pure HBM-write bandwidth: stage one super-row per SBUF
partition (all 128 partitions identical content, phase-shifted per core
on the host), then blast it to the output DRAM with a few large DMAs.

Layout: SBUF tile [128, 6144] f32, partition p supplies output rows
[c*1536 + p*12, c*1536 + p*12 + 12) of each 1536-row chunk. Since
1536 % 12 == 0 the same tile serves every chunk.
"""

import numpy as np

EMBED_DIM = 512
D4 = EMBED_DIM // 4
NUM_FEATURES = 65536
N_CORES = 8
ROWS_PER_CORE = NUM_FEATURES // N_CORES  # 8192
PERIOD = 12                              # lcm(3, 4)
SUPER = PERIOD * EMBED_DIM               # 6144 f32 per partition
CHUNK_ROWS = 128 * PERIOD                # 1536 rows per full-tile store
N_FULL = ROWS_PER_CORE // CHUNK_ROWS     # 5 full chunks -> 7680 rows
REM_ROWS = ROWS_PER_CORE - N_FULL * CHUNK_ROWS   # 512
REM_PARTS = REM_ROWS // PERIOD           # 42 partitions -> 504 rows
TAIL_ROWS = REM_ROWS - REM_PARTS * PERIOD  # 8 rows

_CACHE = {}

# test.py hooks (harness ignores these)
TRACE = False
LAST_EXEC_NS = None
LAST_RESULTS = None


def _build_program():
    import concourse.bass as bass
    import concourse.bacc as bacc
    import concourse.mybir as mybir

    nc = bacc.Bacc(
        "TRN2",
        target_bir_lowering=False,
        debug=False,
        enable_asserts=False,
        num_devices=N_CORES,
    )
    f32 = mybir.dt.float32
    block = nc.dram_tensor("block", [128, SUPER], f32, kind="ExternalInput").ap()
    out = nc.dram_tensor("out", [ROWS_PER_CORE, EMBED_DIM], f32, kind="ExternalOutput").ap()

    NPIECE = 4
    PW = SUPER // NPIECE  # 1536 cols = 3 rows per partition per piece
    with (
        nc.sbuf_tensor("pat", [128, SUPER], f32) as t,
        nc.semaphore("ld_sem") as ld_sem,
        nc.semaphore("st_sem") as st_sem,
        nc.Block() as blk,
    ):
        rem = out[N_FULL * CHUNK_ROWS : N_FULL * CHUNK_ROWS + REM_PARTS * PERIOD]
        rem = rem.rearrange("(p r) d -> p (r d)", r=PERIOD)  # [42, SUPER]
        tail = out[ROWS_PER_CORE - TAIL_ROWS :].rearrange(
            "(p r) d -> p (r d)", p=1
        )  # [1, TAIL_ROWS*512]
        full = out[: N_FULL * CHUNK_ROWS].rearrange(
            "(k p r) d -> k p (r d)", p=128, r=PERIOD
        )  # [N_FULL, 128, SUPER]
        N_STORES = NPIECE + (N_FULL - 1) + 2

        # Load arrives in NPIECE column pieces; chunk-0 stores chase the
        # pieces so the write stream starts after the first piece lands.
        # Two HWDGE rings (SP + ACT) double descriptor-feed rate.
        @blk.sync
        def _(sync):
            for i in range(NPIECE):
                sync.dma_start(
                    t[:, i * PW : (i + 1) * PW], block[:, i * PW : (i + 1) * PW]
                ).then_inc(ld_sem, 16)
            for i in (0, 1):
                sync.wait_ge(ld_sem, 16 * (i + 1))
                sync.dma_start(full[0][:, i * PW : (i + 1) * PW], t[:, i * PW : (i + 1) * PW]).then_inc(st_sem, 16)
            sync.wait_ge(ld_sem, 16 * NPIECE)
            for k in (2, 4):
                sync.dma_start(full[k], t[:, :]).then_inc(st_sem, 16)
            sync.dma_start(tail, t[:1, : TAIL_ROWS * EMBED_DIM]).then_inc(st_sem, 16)
            sync.wait_ge(st_sem, 16 * N_STORES)

        @blk.scalar
        def _(scalar):
            for i in (2, 3):
                scalar.wait_ge(ld_sem, 16 * (i + 1))
                scalar.dma_start(full[0][:, i * PW : (i + 1) * PW], t[:, i * PW : (i + 1) * PW]).then_inc(st_sem, 16)
            for k in (1, 3):
                scalar.dma_start(full[k], t[:, :]).then_inc(st_sem, 16)
            scalar.dma_start(rem, t[0 : 3 * REM_PARTS : 3, :]).then_inc(st_sem, 16)
    nc.compile()
    return nc


def _get_program():
    if "nc" not in _CACHE:
        _CACHE["nc"] = _build_program()
    return _CACHE["nc"]


def _host_blocks(price_w, size_w, exchange_w, pair_w):
    """Per-core [128, SUPER] f32 pattern blocks (all partitions identical)."""
    idx = np.arange(PERIOD)
    row12 = np.concatenate(
        [
            np.broadcast_to(price_w[0], (PERIOD, D4)),
            np.broadcast_to(size_w[0], (PERIOD, D4)),
            exchange_w[idx % 3],
            pair_w[idx % 4],
        ],
        axis=-1,
    ).astype(np.float32)  # [12, 512]
    blocks = []
    for c in range(N_CORES):
        base = c * ROWS_PER_CORE
        s = row12[(base + idx) % PERIOD].reshape(-1)  # [SUPER]
        blocks.append(np.ascontiguousarray(np.broadcast_to(s, (128, SUPER))))
    return blocks


def kernel(num_features, price_w, size_w, exchange_w, pair_w):
    global LAST_EXEC_NS, LAST_RESULTS
    from concourse.bass_utils import run_bass_kernel_spmd

    assert int(num_features) == NUM_FEATURES
    price_w = np.asarray(price_w, dtype=np.float32)
    size_w = np.asarray(size_w, dtype=np.float32)
    exchange_w = np.asarray(exchange_w, dtype=np.float32)
    pair_w = np.asarray(pair_w, dtype=np.float32)

    nc = _get_program()
    in_maps = [{"block": b} for b in _host_blocks(price_w, size_w, exchange_w, pair_w)]
    res = run_bass_kernel_spmd(nc, in_maps, list(range(N_CORES)), trace=TRACE)
    LAST_EXEC_NS = res.exec_time_ns
    LAST_RESULTS = res
    return np.concatenate([res.results[c]["out"] for c in range(N_CORES)], axis=0)
